# revision 8
# baseline (speedup 1.0000x reference)
"""MoE FFN (dense masked-accumulate) for 8 Trainium2 NeuronCores.

Strategy (expert-parallel, mirroring the module's dist layout):
  - Router computed on host (0.01% of FLOPs): softmax(x@Wr), top-2,
    renormalize, scatter -> dispatch[B*T, E].
  - Each core e computes its expert's dense FFN over ALL tokens:
        hT = gelu(W1[e].T-style matmul)  -> [H, M] layout in DRAM
        y  = (hT.T @ W2[e]) * disp[:, e] -> [M, D]
  - Host sums the 8 per-core partials (the "all-reduce") and adds the
    b2 term (sum_e disp_e = 1 after renormalization).

Device layout convention: a logical [R, C] matrix is stored in DRAM as
[128, R/128, C] with row r -> [r % 128, r // 128, :]  (partition-inner).

Per-core phases:
  Phase 1 (fc1): lhsT = W1 tile [d_k=128, h=128] (stationary),
                 rhs  = xT tile [d_k=128, m=512] (moving)
                 psum [h=128, m=512] accumulated over 8 d_k tiles,
                 gelu(+b1) evict -> hT DRAM [128, 32, 8192].
  Phase 2 (fc2): lhsT = hT tile [h_k=128, m=128] (stationary),
                 rhs  = W2 tile [h_k=128, d=512] (moving)
                 psum [m=128, d=512] accumulated over 32 h_k tiles,
                 * disp (per-partition scalar) evict -> out DRAM.
"""

import os
import sys
import numpy as np

if "/opt/trn_rl_repo" not in sys.path:
    sys.path.insert(0, "/opt/trn_rl_repo")

# Problem dims (hardcoded per contract).
B, T, D, H, E, TOPK = 2, 4096, 1024, 4096, 8, 2
M = B * T  # 8192 tokens
NCORES = 8
P = 128

_CACHE = {}
LAST_EXEC_TIME_NS = None
LAST_RESULTS = None

# Sparse path: per-expert token capacity (real input peaks at 2182).
CAP = 2560


def _route_host(x2, Wr):
    """Host router: returns dispatch [M, E] float32 (top-2 renormalized)."""
    logits = x2 @ Wr  # [M, E] fp32
    logits = logits - logits.max(axis=-1, keepdims=True)
    p = np.exp(logits)
    p = p / p.sum(axis=-1, keepdims=True)
    # top-2 of E=8
    a1 = np.argmax(p, axis=-1)
    rows = np.arange(p.shape[0])
    p1 = p[rows, a1]
    p_masked = p.copy()
    p_masked[rows, a1] = -np.inf
    a2 = np.argmax(p_masked, axis=-1)
    p2 = p_masked[rows, a2]
    s = p1 + p2
    disp = np.zeros_like(p)
    disp[rows, a1] = p1 / s
    disp[rows, a2] = p2 / s
    return disp.astype(np.float32)


def _pm(a2d):
    """[R, C] -> [128, R/128, C] with row r -> [r%128, r//128]."""
    R, C = a2d.shape
    return np.ascontiguousarray(a2d.reshape(R // P, P, C).transpose(1, 0, 2))


def _build_nc(mm_dt_name, M=M):
    import concourse.bass as bass
    import concourse.bacc as bacc
    import concourse.mybir as mybir
    from concourse.tile import TileContext

    mm_dt = getattr(mybir.dt, mm_dt_name)
    f32 = mybir.dt.float32

    KD = D // P        # 8   d_k tiles
    KH = H // P        # 32  h_k tiles
    NMB = M // 512     # 16  token blocks (phase 1)
    NHC = H // P       # 32  h chunks (phase 1)
    NMT = M // P       # 64  token tiles (phase 2)
    NDB = D // 512     # 2   d blocks (phase 2)

    nc = bacc.Bacc(None, target_bir_lowering=False, debug=False)

    xT = nc.dram_tensor("xT", [P, KD, M], mm_dt, kind="ExternalInput")
    w1 = nc.dram_tensor("w1", [P, KD, H], mm_dt, kind="ExternalInput")
    w2 = nc.dram_tensor("w2", [P, KH, D], mm_dt, kind="ExternalInput")
    b1t = nc.dram_tensor("b1t", [P, NHC], f32, kind="ExternalInput")
    dsp = nc.dram_tensor("dsp", [P, NMT], f32, kind="ExternalInput")
    out = nc.dram_tensor("out", [P, NMT, D], f32, kind="ExternalOutput")

    with TileContext(nc) as tc:
        with tc.tile_pool(name="dram", bufs=1, space="DRAM") as dram, \
             tc.tile_pool(name="const", bufs=1) as const:
            # Intermediate hT, one DRAM tile per token block so phase 2
            # token tiles only depend on their own block's fc1 writes.
            hT_blocks = [
                dram.tile([P, NHC, 512], mm_dt, name=f"hT{mb}")
                for mb in range(NMB)
            ]
            b1_sb = const.tile([P, NHC], f32, name="b1_sb")
            nc.sync.dma_start(b1_sb[:], b1t[:])
            dsp_sb = const.tile([P, NMT], f32, name="dsp_sb")
            nc.sync.dma_start(dsp_sb[:], dsp[:])

            # ---- Phase 1: hT = gelu(x @ W1 + b1), stored [H, M] ----
            with tc.tile_pool(name="w1p", bufs=1) as w1p, \
                 tc.tile_pool(name="xp", bufs=3) as xp, \
                 tc.tile_pool(name="hp", bufs=6) as hp, \
                 tc.tile_pool(name="ps1", bufs=4, space="PSUM") as ps1:
                # W1 resident, split per h-chunk for fine-grained deps.
                w1_sb = []
                for hc in range(NHC):
                    t = w1p.tile([P, KD, P], mm_dt, name=f"w1c{hc}")
                    nc.sync.dma_start(t[:], w1[:, :, hc * P:(hc + 1) * P])
                    w1_sb.append(t)
                for mb in range(NMB):
                    x_sb = xp.tile([P, KD, 512], mm_dt, name="x_sb")
                    nc.sync.dma_start(x_sb[:], xT[:, :, mb * 512:(mb + 1) * 512])
                    for hc in range(NHC):
                        psum = ps1.tile([P, 512], f32, name="ps1t")
                        for k in range(KD):
                            nc.tensor.matmul(
                                psum[:],
                                lhsT=w1_sb[hc][:, k:k + 1, :],
                                rhs=x_sb[:, k:k + 1, :],
                                start=(k == 0),
                                stop=(k == KD - 1),
                            )
                        h_sb = hp.tile([P, 512], mm_dt, name="h_sb")
                        nc.scalar.activation(
                            h_sb[:], psum[:],
                            mybir.ActivationFunctionType.Gelu,
                            bias=b1_sb[:, hc:hc + 1],
                        )
                        nc.sync.dma_start(hT_blocks[mb][:, hc, :], h_sb[:])

            # ---- Phase 2: out = (hT.T @ W2) * disp ----
            with tc.tile_pool(name="w2p", bufs=1) as w2p, \
                 tc.tile_pool(name="hp2", bufs=3) as hp2, \
                 tc.tile_pool(name="op", bufs=6) as op, \
                 tc.tile_pool(name="ps2", bufs=4, space="PSUM") as ps2:
                w2_sb = []
                for k in range(KH):
                    t = w2p.tile([P, 1, D], mm_dt, name=f"w2c{k}")
                    nc.sync.dma_start(t[:], w2[:, k:k + 1, :])
                    w2_sb.append(t)
                for mt in range(NMT):
                    mb, off = mt // 4, (mt % 4) * P
                    hT_sb = hp2.tile([P, KH, P], mm_dt, name="hT_sb")
                    nc.sync.dma_start(
                        hT_sb[:], hT_blocks[mb][:, :, off:off + P])
                    for db in range(NDB):
                        psum = ps2.tile([P, 512], f32, name="ps2t")
                        for k in range(KH):
                            nc.tensor.matmul(
                                psum[:],
                                lhsT=hT_sb[:, k:k + 1, :],
                                rhs=w2_sb[k][:, :, db * 512:(db + 1) * 512],
                                start=(k == 0),
                                stop=(k == KH - 1),
                            )
                        o_sb = op.tile([P, 512], f32, name="o_sb")
                        nc.vector.tensor_scalar_mul(
                            o_sb[:], psum[:], dsp_sb[:, mt:mt + 1])
                        nc.sync.dma_start(
                            out[:, mt, db * 512:(db + 1) * 512], o_sb[:])

    nc.compile()
    return nc


def _get_nc(m_tokens=M):
    mm_dt_name = os.environ.get("KERNEL_MM_DT", "float32")
    key = ("nc", mm_dt_name, m_tokens)
    if key not in _CACHE:
        _CACHE[key] = _build_nc(mm_dt_name, M=m_tokens)
    return _CACHE[key]


def bench_spmd(nc, in_maps, iters=5):
    """Time repeated on-device executions with device-resident inputs.

    Replicates bass2jax.run_bass_via_pjrt's sharded invocation, puts the
    concatenated inputs on device once, and wall-clocks the jitted call.
    Returns (best_seconds, results_of_last_call).
    """
    import time as _time
    import jax
    import jax.numpy as jnp
    from jax.sharding import Mesh, PartitionSpec
    from jax.experimental.shard_map import shard_map
    import concourse.mybir as mybir
    from concourse import bass2jax
    from concourse.bass2jax import _bass_exec_p, install_neuronx_cc_hook

    install_neuronx_cc_hook()
    n_cores = len(in_maps)

    partition_name = (
        nc.partition_id_tensor.name if nc.partition_id_tensor else None)
    in_names, out_names, out_avals = [], [], []
    for alloc in nc.m.functions[0].allocations:
        if not isinstance(alloc, mybir.MemoryLocationSet):
            continue
        name = alloc.memorylocations[0].name
        if alloc.kind == "ExternalInput":
            if name != partition_name:
                in_names.append(name)
        elif alloc.kind == "ExternalOutput":
            out_names.append(name)
            out_avals.append(jax.core.ShapedArray(
                tuple(alloc.tensor_shape), mybir.dt.np(alloc.dtype)))
    n_params = len(in_names)
    n_outs = len(out_avals)
    all_in_names = in_names + out_names
    if partition_name is not None:
        all_in_names = all_in_names + [partition_name]

    def _body(*args):
        operands = list(args)
        if partition_name is not None:
            operands.append(bass2jax.partition_id_tensor())
        outs = _bass_exec_p.bind(
            *operands,
            out_avals=tuple(out_avals),
            in_names=tuple(all_in_names),
            out_names=tuple(out_names),
            lowering_input_output_aliases=(),
            sim_require_finite=True,
            sim_require_nnan=True,
            nc=nc,
        )
        return tuple(outs)

    devices = jax.devices()[:n_cores]
    mesh = Mesh(np.asarray(devices), ("core",))
    donate = tuple(range(n_params, n_params + n_outs))
    sharded = jax.jit(
        shard_map(_body, mesh=mesh,
                  in_specs=(PartitionSpec("core"),) * (n_params + n_outs),
                  out_specs=(PartitionSpec("core"),) * n_outs,
                  check_rep=False),
        donate_argnums=donate, keep_unused=True)

    sh = jax.sharding.NamedSharding(mesh, PartitionSpec("core"))
    dev_in = [
        jax.device_put(
            np.concatenate([np.asarray(m[name]) for m in in_maps], axis=0), sh)
        for name in in_names
    ]

    def _zeros():
        return [
            jax.device_put(
                np.zeros((n_cores * a.shape[0], *a.shape[1:]), a.dtype), sh)
            for a in out_avals
        ]

    # warmup (compiles and runs once)
    out = sharded(*dev_in, *_zeros())
    jax.block_until_ready(out)
    best = float("inf")
    for _ in range(iters):
        z = _zeros()
        jax.block_until_ready(z)
        t0 = _time.perf_counter()
        out = sharded(*dev_in, *z)
        jax.block_until_ready(out)
        best = min(best, _time.perf_counter() - t0)
    results = [
        {name: np.asarray(out[i]).reshape(n_cores, *out_avals[i].shape)[c]
         for i, name in enumerate(out_names)}
        for c in range(n_cores)
    ]
    return best, results


def _core_weight_inputs(W1, b1, W2, e):
    return {
        "w1": _pm(W1[e]),                       # [128, 8, 4096]
        "w2": _pm(W2[e]),                       # [128, 32, 1024]
        "b1t": np.ascontiguousarray(
            b1[e].reshape(H // P, P).T),        # [128, 32]
    }


def _dense_in_maps(x2, disp, W1, b1, W2):
    xT_pm = _pm(np.ascontiguousarray(x2.T))  # [128, 8, 8192]
    in_maps = []
    for e in range(NCORES):
        m = _core_weight_inputs(W1, b1, W2, e)
        m["xT"] = xT_pm
        m["dsp"] = np.ascontiguousarray(disp[:, e].reshape(M // P, P).T)
        in_maps.append(m)
    return in_maps


def _sparse_in_maps(x2, disp, W1, b1, W2):
    """Gather each expert's routed tokens (padded to CAP). Returns
    (in_maps, idx_list) or None if any expert overflows CAP."""
    in_maps, idx_list = [], []
    for e in range(NCORES):
        idx = np.nonzero(disp[:, e] > 0)[0]
        if idx.size > CAP:
            return None
        x_e = np.zeros((CAP, D), dtype=np.float32)
        x_e[:idx.size] = x2[idx]
        d_e = np.zeros((CAP,), dtype=np.float32)
        d_e[:idx.size] = disp[idx, e]
        m = _core_weight_inputs(W1, b1, W2, e)
        m["xT"] = _pm(np.ascontiguousarray(x_e.T))   # [128, 8, CAP]
        m["dsp"] = np.ascontiguousarray(d_e.reshape(CAP // P, P).T)
        in_maps.append(m)
        idx_list.append(idx)
    return in_maps, idx_list


def _run_spmd(nc, in_maps):
    from concourse import bass_utils
    res = bass_utils.run_bass_kernel_spmd(
        nc, in_maps, core_ids=list(range(NCORES)))
    return res.results


def kernel(x, Wr, W1, b1, W2, b2):
    global LAST_RESULTS

    x2 = np.ascontiguousarray(np.asarray(x, dtype=np.float32).reshape(M, D))
    Wr = np.asarray(Wr, dtype=np.float32)
    W1 = np.asarray(W1, dtype=np.float32)
    b1 = np.asarray(b1, dtype=np.float32)
    W2 = np.asarray(W2, dtype=np.float32)
    b2 = np.asarray(b2, dtype=np.float32)

    disp = _route_host(x2, Wr)  # [M, E]
    mode = os.environ.get("KERNEL_MODE", "auto")

    sparse = None
    if mode in ("auto", "sparse"):
        sparse = _sparse_in_maps(x2, disp, W1, b1, W2)
    if sparse is not None:
        in_maps, idx_list = sparse
        nc = _get_nc(CAP)
        results = _run_spmd(nc, in_maps)
        LAST_RESULTS = results
        out2 = np.zeros((M, D), dtype=np.float32)
        for e in range(NCORES):
            y = results[e]["out"].transpose(1, 0, 2).reshape(CAP, D)
            out2[idx_list[e]] += y[:idx_list[e].size]
    else:
        in_maps = _dense_in_maps(x2, disp, W1, b1, W2)
        nc = _get_nc(M)
        results = _run_spmd(nc, in_maps)
        LAST_RESULTS = results
        acc = np.zeros((P, M // P, D), dtype=np.float32)
        for r in results:
            acc += r["out"]
        out2 = acc.transpose(1, 0, 2).reshape(M, D)

    out2 = out2 + disp @ b2  # sum_e disp_e * b2[e]
    return out2.reshape(B, T, D)
